# revision 10
# baseline (speedup 1.0000x reference)
"""Augmented Neural ODE kernel for 8 TRN2 NeuronCores — fp8 SwInterleave variant.

Data-parallel over the batch dim (8 batches/core -> 512 tokens/core);
state kept feature-major [STATE=128 partitions, 512 tokens] in SBUF.
Layers 1-3 (contraction 1024) run as fp8e4m3 DoubleRowSwInterleave
matmuls: weights pre-interleaved host-side ([A127,B127,A126,B126,...]
per partition) so LDWEIGHTS reads contiguously — plain DoubleRow's
interleaved gather costs ~35ns/MM of PE-array time on top of the 216ns
N=512 stream; SwInterleave removes it (measured 251 -> 220 ns/MM).
Layer 0 runs in f32r straight off the carry (K=128 can't DoubleRow);
the Euler carry y' = y + dt*f rides layer 3's PSUM group via an
s3-scaled f32r identity matmul (power-of-two scale, lossless).

Matmuls are emitted in a hand-scheduled order (L1SEQ): layer 0's eight
f32r matmuls first, then layer 1/2 groups interleaved so (a) each
group's first matmul lands just after its PSUM bank is freed by the
previous layer's tanh, (b) k-chunks are consumed just after the
serial tanh chain (24 ACTs x ~593ns, scalar engine) produces them, and
(c) groups complete spread out so the next layer's ACT chain starts
early. This keeps the PE within ~8% of its 77-pass x 216ns/step floor.
"""

import sys

if "/opt/trn_rl_repo" not in sys.path:
    sys.path.insert(0, "/opt/trn_rl_repo")

import numpy as np

B, S, DIN, DAUG = 64, 64, 64, 64
STATE = DIN + DAUG          # 128
HID = 1024
T = 32
NCORES = 8
BSHARD = B // NCORES        # 8
NTOK = BSHARD * S           # 512 tokens per core
KC = HID // 128             # 8 chunks of the hidden dim
KP = KC // 2                # 4 chunk-pairs for DoubleRow

# Interleaved (group, kchunk) emission orders. Layer 1's inputs h0[k]
# arrive late (the L0 tanh chain produces pair k at ~y+1.3us+1.16k us),
# so its k3 matmuls start at slot 15 and group-closes spread to the end;
# its PSUM banks also free at tanh-chain rate, pacing the k0 allocs.
# Layer 2's inputs are all ready by ~1.2us into its stream, so groups
# close round-robin from slot 8 on — the h2 tanh chain then finishes
# just as the PE reaches layer 3's last matmul.
L1SEQ = [(0, 0), (1, 0), (2, 0), (3, 0), (0, 1), (1, 1), (2, 1), (4, 0),
         (3, 1), (5, 0), (0, 2), (1, 2), (4, 1), (6, 0), (2, 2), (0, 3),
         (1, 3), (7, 0), (3, 2), (2, 3), (5, 1), (6, 1), (4, 2), (3, 3),
         (7, 1), (5, 2), (4, 3), (6, 2), (5, 3), (7, 2), (6, 3), (7, 3)]
L2SEQ = [(0, 0), (0, 1), (1, 0), (0, 2), (1, 1), (1, 2), (2, 0), (2, 1),
         (0, 3), (1, 3), (2, 2), (2, 3), (3, 0), (3, 1), (3, 2), (3, 3),
         (4, 0), (4, 1), (4, 2), (4, 3), (5, 0), (5, 1), (5, 2), (5, 3),
         (6, 0), (6, 1), (6, 2), (6, 3), (7, 0), (7, 1), (7, 2), (7, 3)]
for _seq in (L1SEQ, L2SEQ):
    assert sorted(_seq) == [(g, k) for g in range(KC) for k in range(KP)]
    for _g in range(KC):
        _ks = [k for g, k in _seq if g == _g]
        assert _ks == sorted(_ks)

DELAY_COLS = [512, 512]

_cached = {}


def _build(scales):
    """scales = (s1, s2, s3) power-of-two per-matrix weight scales."""
    if scales in _cached:
        return _cached[scales]
    s1, s2, s3 = scales

    import concourse.tile as tile
    from concourse import bacc, mybir

    # The Tile scheduler's cost sim prices DoubleRow/SwInterleave matmuls
    # at 0.5 cycles/row, but the hardware streams one rhs column per cycle
    # regardless (N=512 -> ~216ns, not ~107ns). The mispriced sim commits a
    # per-engine order where all k0-k2 matmuls run before any group closes,
    # so the tanh chains start ~4us late and the PE stalls at every layer
    # boundary. pe_cycle_scale is a settable knob on the sim state that
    # scales only matmul cost; 2.0 makes fp8-DR matmuls exact and lets the
    # scheduler see the real PE-bound timeline.
    import concourse.bass_interp as bass_interp
    if not getattr(bass_interp.CoreSim, "_pe_scale_patched", False):
        _orig_init = bass_interp.CoreSim.__init__

        def _init_with_scale(self, *a, **kw):
            _orig_init(self, *a, **kw)
            try:
                self._sim_state.pe_cycle_scale = 2.0
            except Exception:
                pass

        bass_interp.CoreSim.__init__ = _init_with_scale
        bass_interp.CoreSim._pe_scale_patched = True

    f32 = mybir.dt.float32
    f32r = mybir.dt.float32r
    fp8 = mybir.dt.float8e4
    SWI = mybir.MatmulPerfMode.DoubleRowSwInterleave
    Tanh = mybir.ActivationFunctionType.Tanh
    mult = mybir.AluOpType.mult
    add = mybir.AluOpType.add

    nc = bacc.Bacc("TRN2", target_bir_lowering=False, debug=False,
                   num_devices=NCORES)

    y0t_d = nc.dram_tensor("y0t", [DIN, NTOK], f32r, kind="ExternalInput").ap()
    laug_d = nc.dram_tensor("laug", [DIN, STATE], f32r, kind="ExternalInput").ap()
    w0t_d = nc.dram_tensor("w0t", [STATE, HID], f32r, kind="ExternalInput").ap()
    w1s_d = nc.dram_tensor("w1s", [128, KP, KC, 2, 128], fp8, kind="ExternalInput").ap()
    w2s_d = nc.dram_tensor("w2s", [128, KP, KC, 2, 128], fp8, kind="ExternalInput").ap()
    w3s_d = nc.dram_tensor("w3s", [128, KP, 2, STATE], fp8, kind="ExternalInput").ap()
    idt_d = nc.dram_tensor("idt", [STATE, STATE], f32r, kind="ExternalInput").ap()
    b0_d = nc.dram_tensor("b0", [128, KC], f32, kind="ExternalInput").ap()
    b1_d = nc.dram_tensor("b1", [128, KC], f32, kind="ExternalInput").ap()
    b2_d = nc.dram_tensor("b2", [128, KC], f32, kind="ExternalInput").ap()
    bias2_d = nc.dram_tensor("bias2", [STATE, 2], f32, kind="ExternalInput").ap()
    out_d = nc.dram_tensor("out", [DIN, NTOK], f32r, kind="ExternalOutput").ap()

    with tile.TileContext(nc) as tc:
        with tc.tile_pool(name="wpool", bufs=1) as wpool, \
             tc.tile_pool(name="hpool", bufs=12) as hpool, \
             tc.tile_pool(name="ypool", bufs=2) as ypool, \
             tc.tile_pool(name="pspool", bufs=8, space="PSUM") as pspool:

            # -- staged DMAs, ordered by first use within each queue --
            bias2 = wpool.tile([STATE, 2], f32)
            nc.sync.dma_start(bias2[:], bias2_d[:])
            laug = wpool.tile([DIN, STATE], f32r)
            nc.sync.dma_start(laug[:], laug_d[:])
            y0t = wpool.tile([DIN, NTOK], f32r)
            nc.sync.dma_start(y0t[:], y0t_d[:])
            b0 = wpool.tile([128, KC], f32)
            nc.sync.dma_start(b0[:], b0_d[:])

            w0t = wpool.tile([128, HID], f32r)
            nc.gpsimd.dma_start(w0t[:, 0:HID // 2], w0t_d[:, 0:HID // 2])
            nc.scalar.dma_start(w0t[:, HID // 2:], w0t_d[:, HID // 2:])

            w1s = wpool.tile([128, KP, KC, 2, 128], fp8)
            nc.sync.dma_start(w1s[:, 0], w1s_d[:, 0])
            nc.gpsimd.dma_start(w1s[:, 1], w1s_d[:, 1])
            nc.scalar.dma_start(w1s[:, 2], w1s_d[:, 2])
            nc.gpsimd.dma_start(w1s[:, 3], w1s_d[:, 3])

            b1 = wpool.tile([128, KC], f32)
            nc.sync.dma_start(b1[:], b1_d[:])
            b2 = wpool.tile([128, KC], f32)
            nc.sync.dma_start(b2[:], b2_d[:])

            w2s = wpool.tile([128, KP, KC, 2, 128], fp8)
            nc.sync.dma_start(w2s[:, 0], w2s_d[:, 0])
            nc.gpsimd.dma_start(w2s[:, 1], w2s_d[:, 1])
            nc.scalar.dma_start(w2s[:, 2], w2s_d[:, 2])
            nc.sync.dma_start(w2s[:, 3], w2s_d[:, 3])

            w3s = wpool.tile([128, KP, 2, STATE], fp8)
            nc.gpsimd.dma_start(w3s[:], w3s_d[:])
            idt = wpool.tile([STATE, STATE], f32r)
            nc.sync.dma_start(idt[:], idt_d[:])

            # -- augment: y = [y0; W_aug y0] + baug   (K = 64, one-time) --
            ps = pspool.tile([128, NTOK], f32, tag="ps")
            nc.tensor.matmul(ps[:], lhsT=laug[:], rhs=y0t[:],
                             start=True, stop=True)
            y = ypool.tile([128, NTOK], f32r, tag="y")
            nc.vector.tensor_scalar(y[:], ps[:], 1.0, bias2[:, 0:1], mult, add)

            for n in range(T - 1):
                # layer 0: f32r off the carry, K=128
                h0 = [hpool.tile([128, 2, NTOK], fp8, tag="h",
                                 name=f"h0_{n}_{i}") for i in range(KP)]
                for m in range(KC):
                    ps = pspool.tile([128, NTOK], f32, tag="ps")
                    nc.tensor.matmul(ps[:], lhsT=w0t[:, m * 128:(m + 1) * 128],
                                     rhs=y[:], start=True, stop=True)
                    nc.scalar.activation(h0[m // 2][:, m % 2, :], ps[:], Tanh,
                                         bias=b0[:, m:m + 1])

                # layers 1 and 2: fp8 SwInterleave, K=256 per matmul
                def big_layer(ws, hin, bias, scale, hname, seq):
                    hout = [hpool.tile([128, 2, NTOK], fp8, tag="h",
                                       name=f"{hname}_{n}_{i}")
                            for i in range(KP)]
                    psg = {}
                    for g, k in seq:
                        if k == 0:
                            psg[g] = pspool.tile([128, NTOK], f32, tag="ps",
                                                 name=f"{hname}ps_{n}_{g}")
                        nc.tensor.matmul(psg[g][:], lhsT=ws[:, k, g],
                                         rhs=hin[k][:],
                                         start=(k == 0), stop=(k == KP - 1),
                                         perf_mode=SWI)
                        if k == KP - 1:
                            nc.scalar.activation(hout[g // 2][:, g % 2, :],
                                                 psg[g][:], Tanh,
                                                 bias=bias[:, g:g + 1],
                                                 scale=scale)
                    return hout

                h1 = big_layer(w1s, h0, b1, 1.0 / s1, "h1", L1SEQ)
                h2 = big_layer(w2s, h1, b2, 1.0 / s2, "h2", L2SEQ)

                # layer 3 + Euler carry: ps3 = s3*y + s3*dt*W3 h2
                ps3 = pspool.tile([128, NTOK], f32, tag="ps",
                                  name=f"ps3_{n}")
                nc.tensor.matmul(ps3[:], lhsT=idt[:], rhs=y[:],
                                 start=True, stop=False)
                for j in range(KP):
                    nc.tensor.matmul(ps3[:], lhsT=w3s[:, j], rhs=h2[j][:],
                                     start=False, stop=(j == KP - 1),
                                     perf_mode=SWI)

                if n < T - 2:
                    y = ypool.tile([128, NTOK], f32r, tag="y")
                    nc.vector.tensor_scalar(y[:], ps3[:], 1.0 / s3,
                                            bias2[:, 1:2], mult, add)
                    # power pacing: stretch the carry->L0 serial path so the
                    # kernel's average power stays under the P0 threshold
                    # (a denser schedule downclocks every engine to 5/6).
                    for w in DELAY_COLS:
                        nc.vector.tensor_scalar(y[:, 0:w], y[:, 0:w], 1.0,
                                                0.0, mult, add)
                else:
                    yout = ypool.tile([128, NTOK], f32r, tag="y")
                    nc.vector.tensor_scalar(yout[0:DIN, :], ps3[0:DIN, :],
                                            1.0 / s3, bias2[0:DIN, 1:2],
                                            mult, add)

            nc.sync.dma_start(out_d[:], yout[0:DIN, :])

    nc.compile()
    _cached[scales] = nc
    return nc


def _pow2_scale(W, target=224.0):
    import math
    return 2.0 ** math.floor(math.log2(target / float(np.abs(W).max())))


def _swi_pairs(Wt):
    """Wt: [K_in, M_out] (lhsT orientation) with K_in = 256*kp.
    Returns [128, kp, M_out//128, 2, 128] in SwInterleave layout:
    per partition the 256 weights of a (k, m) chunk are
    [A_{127}, B_{127}, A_{126}, B_{126}, ..., A_0, B_0] where A/B are
    K-subchunks 2k / 2k+1 and columns run in reverse."""
    K_in, M_out = Wt.shape
    kp = K_in // 256
    mc = M_out // 128
    out = np.empty((128, kp, mc, 2, 128), np.float32)
    for k in range(kp):
        lo = Wt[(2 * k) * 128:(2 * k + 1) * 128]       # [128, M_out]
        hi = Wt[(2 * k + 1) * 128:(2 * k + 2) * 128]
        for m in range(mc):
            ms = slice(m * 128, (m + 1) * 128)
            pair = np.stack([lo[:, ms], hi[:, ms]], axis=1)  # [128, 2, 128]
            tmp = pair[:, :, ::-1].transpose(0, 2, 1)        # [128, 128, 2]
            out[:, k, m] = tmp.reshape(128, 2, 128)
    return out


def _make_in_maps(y0, t, W_aug, b_aug, W0, b0, W1, b1, W2, b2, W3, b3):
    import ml_dtypes
    f = np.float32
    f8 = ml_dtypes.float8_e4m3
    dt = float(np.asarray(t, dtype=f)[1] - np.asarray(t, dtype=f)[0])
    W1, W2 = np.asarray(W1, f), np.asarray(W2, f)
    W3dt = dt * np.asarray(W3, f)
    s1, s2, s3 = _pow2_scale(W1), _pow2_scale(W2), _pow2_scale(W3dt)

    laug = np.concatenate([np.eye(DIN, dtype=f),
                           np.asarray(W_aug, f).T], axis=1)
    w0t = np.ascontiguousarray(np.asarray(W0, f).T)
    w1s = np.ascontiguousarray(_swi_pairs((W1 * s1).T)).astype(f8)
    w2s = np.ascontiguousarray(_swi_pairs((W2 * s2).T)).astype(f8)
    w3s = np.ascontiguousarray(
        _swi_pairs((W3dt * s3).T)[:, :, 0]).astype(f8)  # [128, KP, 2, 128]
    idt = np.eye(STATE, dtype=f) * s3
    b0r = np.ascontiguousarray(np.asarray(b0, f).reshape(KC, 128).T)
    b1r = np.ascontiguousarray(np.asarray(b1, f).reshape(KC, 128).T)
    b2r = np.ascontiguousarray(np.asarray(b2, f).reshape(KC, 128).T)
    baug_full = np.concatenate([np.zeros(DIN, f), np.asarray(b_aug, f)])
    b3dt = dt * np.asarray(b3, f)
    bias2 = np.ascontiguousarray(np.stack([baug_full, b3dt], axis=1))

    shared = dict(laug=laug, w0t=w0t, w1s=w1s, w2s=w2s, w3s=w3s, idt=idt,
                  b0=b0r, b1=b1r, b2=b2r, bias2=bias2)
    in_maps = []
    for c in range(NCORES):
        y0c = np.ascontiguousarray(
            np.asarray(y0, np.float32)[c * BSHARD:(c + 1) * BSHARD]
            .reshape(NTOK, DIN).T)
        in_maps.append(dict(y0t=y0c, **shared))
    return in_maps, (s1, s2, s3)


def _run(inputs, trace=False, **trace_kwargs):
    from concourse.bass_utils import run_bass_kernel_spmd

    in_maps, scales = _make_in_maps(**inputs)
    nc = _build(scales)
    res = run_bass_kernel_spmd(nc, in_maps, core_ids=list(range(NCORES)),
                               trace=trace, **trace_kwargs)
    outs = [res.results[c]["out"] for c in range(NCORES)]
    full = np.concatenate(
        [o.T.reshape(BSHARD, S, DIN) for o in outs], axis=0)
    return np.ascontiguousarray(full, dtype=np.float32), res


def kernel(**inputs):
    out, _ = _run(inputs, trace=False)
    return out


# revision 11
# speedup vs baseline: 1.1895x; 1.1895x over previous
"""Augmented Neural ODE kernel for 8 TRN2 NeuronCores — fp8 SwInterleave variant.

Data-parallel over the batch dim (8 batches/core -> 512 tokens/core);
state kept feature-major [STATE=128 partitions, 512 tokens] in SBUF.
Layers 1-3 (contraction 1024) run as fp8e4m3 DoubleRowSwInterleave
matmuls: weights pre-interleaved host-side ([A127,B127,A126,B126,...]
per partition) so LDWEIGHTS reads contiguously — plain DoubleRow's
gather pays ~35ns/MM of extra PE-array time on top of the 216ns N=512
stream; SwInterleave removes it (measured 251 -> 220 ns/MM back-to-back).
Layer 0 runs in f32r straight off the carry (K=128 can't DoubleRow).
The Euler carry y' = y + dt*f rides layer 3's PSUM accumulation group
via an s3-scaled f32r identity matmul (s3 is the power-of-two fp8
scale of dt*W3, so the fold is lossless).

Note: the chip's power manager caps sustained density — schedules that
would finish below ~660us get every core-domain clock cut to 5/6 (PE
2.4->2.0GHz), which is a net loss. The m-outer emission order here
measures at full clock; denser hand schedules measured faster-per-clock
but throttled slower end-to-end.
"""

import sys

if "/opt/trn_rl_repo" not in sys.path:
    sys.path.insert(0, "/opt/trn_rl_repo")

import numpy as np

B, S, DIN, DAUG = 64, 64, 64, 64
STATE = DIN + DAUG          # 128
HID = 1024
T = 32
NCORES = 8
BSHARD = B // NCORES        # 8
NTOK = BSHARD * S           # 512 tokens per core
KC = HID // 128             # 8 chunks of the hidden dim
KP = KC // 2                # 4 chunk-pairs for DoubleRow

_cached = {}


def _build(scales):
    """scales = (s1, s2, s3) power-of-two per-matrix weight scales."""
    if scales in _cached:
        return _cached[scales]
    s1, s2, s3 = scales

    import concourse.tile as tile
    from concourse import bacc, mybir

    f32 = mybir.dt.float32
    f32r = mybir.dt.float32r
    fp8 = mybir.dt.float8e4
    SWI = mybir.MatmulPerfMode.DoubleRowSwInterleave
    Tanh = mybir.ActivationFunctionType.Tanh

    nc = bacc.Bacc("TRN2", target_bir_lowering=False, debug=False,
                   num_devices=NCORES)

    y0t_d = nc.dram_tensor("y0t", [DIN, NTOK], f32r, kind="ExternalInput").ap()
    laug_d = nc.dram_tensor("laug", [DIN, STATE], f32r, kind="ExternalInput").ap()
    baug_d = nc.dram_tensor("baug", [STATE, 1], f32, kind="ExternalInput").ap()
    w0t_d = nc.dram_tensor("w0t", [STATE, HID], f32r, kind="ExternalInput").ap()
    w1s_d = nc.dram_tensor("w1s", [128, KP, KC, 2, 128], fp8, kind="ExternalInput").ap()
    w2s_d = nc.dram_tensor("w2s", [128, KP, KC, 2, 128], fp8, kind="ExternalInput").ap()
    w3s_d = nc.dram_tensor("w3s", [128, KP, 2, STATE], fp8, kind="ExternalInput").ap()
    b0_d = nc.dram_tensor("b0", [128, KC], f32, kind="ExternalInput").ap()
    b1_d = nc.dram_tensor("b1", [128, KC], f32, kind="ExternalInput").ap()
    b2_d = nc.dram_tensor("b2", [128, KC], f32, kind="ExternalInput").ap()
    b3dt_d = nc.dram_tensor("b3dt", [STATE, 1], f32, kind="ExternalInput").ap()
    idt_d = nc.dram_tensor("idt", [STATE, STATE], f32r, kind="ExternalInput").ap()
    out_d = nc.dram_tensor("out", [DIN, NTOK], f32r, kind="ExternalOutput").ap()

    with tile.TileContext(nc) as tc:
        with tc.tile_pool(name="wpool", bufs=1) as wpool, \
             tc.tile_pool(name="hpool", bufs=12) as hpool, \
             tc.tile_pool(name="ypool", bufs=2) as ypool, \
             tc.tile_pool(name="pspool", bufs=8, space="PSUM") as pspool:

            w0t = wpool.tile([128, HID], f32r)
            nc.sync.dma_start(w0t[:], w0t_d[:])
            laug = wpool.tile([DIN, STATE], f32r)
            nc.sync.dma_start(laug[:], laug_d[:])
            y0t = wpool.tile([DIN, NTOK], f32r)
            nc.sync.dma_start(y0t[:], y0t_d[:])

            w1s = wpool.tile([128, KP, KC, 2, 128], fp8)
            nc.sync.dma_start(w1s[:, 0], w1s_d[:, 0])
            nc.gpsimd.dma_start(w1s[:, 1], w1s_d[:, 1])
            nc.scalar.dma_start(w1s[:, 2], w1s_d[:, 2])
            nc.gpsimd.dma_start(w1s[:, 3], w1s_d[:, 3])
            w2s = wpool.tile([128, KP, KC, 2, 128], fp8)
            nc.sync.dma_start(w2s[:, 0], w2s_d[:, 0])
            nc.gpsimd.dma_start(w2s[:, 1], w2s_d[:, 1])
            nc.scalar.dma_start(w2s[:, 2], w2s_d[:, 2])
            nc.sync.dma_start(w2s[:, 3], w2s_d[:, 3])
            w3s = wpool.tile([128, KP, 2, STATE], fp8)
            nc.gpsimd.dma_start(w3s[:], w3s_d[:])
            idt = wpool.tile([STATE, STATE], f32r)
            nc.scalar.dma_start(idt[:], idt_d[:])
            b0 = wpool.tile([128, KC], f32)
            nc.sync.dma_start(b0[:], b0_d[:])
            b1 = wpool.tile([128, KC], f32)
            nc.sync.dma_start(b1[:], b1_d[:])
            b2 = wpool.tile([128, KC], f32)
            nc.sync.dma_start(b2[:], b2_d[:])
            baug = wpool.tile([128, 1], f32)
            nc.sync.dma_start(baug[:], baug_d[:])
            b3dt = wpool.tile([128, 1], f32)
            nc.sync.dma_start(b3dt[:], b3dt_d[:])

            # augment: y = [y0; W_aug y0 + b_aug]   (K = 64, one-time)
            ps = pspool.tile([128, NTOK], f32, tag="ps")
            nc.tensor.matmul(ps[:], lhsT=laug[:], rhs=y0t[:],
                             start=True, stop=True)
            y = ypool.tile([128, NTOK], f32r, tag="y")
            nc.scalar.activation(y[:], ps[:],
                                 mybir.ActivationFunctionType.Identity,
                                 bias=baug[:, 0:1])

            for _step in range(T - 1):
                # layer 0: f32r straight off the carry y
                h0 = [hpool.tile([128, 2, NTOK], fp8, tag="h", name=f"h0_{_step}_{i}")
                      for i in range(KP)]
                for m in range(KC):
                    ps = pspool.tile([128, NTOK], f32, tag="ps")
                    nc.tensor.matmul(ps[:], lhsT=w0t[:, m * 128:(m + 1) * 128],
                                     rhs=y[:], start=True, stop=True)
                    nc.scalar.activation(h0[m // 2][:, m % 2, :], ps[:], Tanh,
                                         bias=b0[:, m:m + 1])
                # layer 1: fp8 SwInterleave, K=256 per matmul
                h1 = [hpool.tile([128, 2, NTOK], fp8, tag="h", name=f"h1_{_step}_{i}")
                      for i in range(KP)]
                for m in range(KC):
                    ps = pspool.tile([128, NTOK], f32, tag="ps")
                    for k in range(KP):
                        nc.tensor.matmul(ps[:], lhsT=w1s[:, k, m],
                                         rhs=h0[k][:],
                                         start=(k == 0), stop=(k == KP - 1),
                                         perf_mode=SWI)
                    nc.scalar.activation(h1[m // 2][:, m % 2, :], ps[:], Tanh,
                                         bias=b1[:, m:m + 1], scale=1.0 / s1)
                # layer 2 with layer 3's matmuls interleaved as their h2
                # pairs become ready; the Euler carry rides the same PSUM
                # group via the s3-scaled f32r identity matmul
                h2 = [hpool.tile([128, 2, NTOK], fp8, tag="h", name=f"h2_{_step}_{i}")
                      for i in range(KP)]
                ps3 = pspool.tile([128, NTOK], f32, tag="ps", name=f"ps3_{_step}")
                nc.tensor.matmul(ps3[:], lhsT=idt[:], rhs=y[:],
                                 start=True, stop=False)
                for m in range(KC):
                    ps = pspool.tile([128, NTOK], f32, tag="ps")
                    for k in range(KP):
                        nc.tensor.matmul(ps[:], lhsT=w2s[:, k, m],
                                         rhs=h1[k][:],
                                         start=(k == 0), stop=(k == KP - 1),
                                         perf_mode=SWI)
                    nc.scalar.activation(h2[m // 2][:, m % 2, :], ps[:], Tanh,
                                         bias=b2[:, m:m + 1], scale=1.0 / s2)
                    if m == 3 or m == 5 or m == 7:
                        k = (m - 3) // 2
                        nc.tensor.matmul(ps3[:], lhsT=w3s[:, k],
                                         rhs=h2[k][:],
                                         start=False, stop=False,
                                         perf_mode=SWI)
                nc.tensor.matmul(ps3[:], lhsT=w3s[:, 3], rhs=h2[3][:],
                                 start=False, stop=True, perf_mode=SWI)
                # carry on the vector engine; scalar stays free for tanhs
                y = ypool.tile([128, NTOK], f32r, tag="y")
                nc.vector.tensor_scalar(y[:], ps3[:], 1.0 / s3, b3dt[:, 0:1],
                                        mybir.AluOpType.mult,
                                        mybir.AluOpType.add)

            nc.sync.dma_start(out_d[:], y[0:DIN, :])

    nc.compile()
    _cached[scales] = nc
    return nc


def _pow2_scale(W, target=224.0):
    import math
    return 2.0 ** math.floor(math.log2(target / float(np.abs(W).max())))


def _swi_pairs(Wt):
    """Wt: [K_in, M_out] (lhsT orientation) with K_in = 256*kp.
    Returns [128, kp, M_out//128, 2, 128] in SwInterleave layout:
    per partition the 256 weights of a (k, m) chunk are
    [A_{127}, B_{127}, ..., A_0, B_0] with A/B = K-subchunks 2k/2k+1
    and columns reversed."""
    K_in, M_out = Wt.shape
    kp = K_in // 256
    mc = M_out // 128
    out = np.empty((128, kp, mc, 2, 128), np.float32)
    for k in range(kp):
        lo = Wt[(2 * k) * 128:(2 * k + 1) * 128]
        hi = Wt[(2 * k + 1) * 128:(2 * k + 2) * 128]
        for m in range(mc):
            ms = slice(m * 128, (m + 1) * 128)
            pair = np.stack([lo[:, ms], hi[:, ms]], axis=1)  # [128, 2, 128]
            tmp = pair[:, :, ::-1].transpose(0, 2, 1)        # [128, 128, 2]
            out[:, k, m] = tmp.reshape(128, 2, 128)
    return out


def _make_in_maps(y0, t, W_aug, b_aug, W0, b0, W1, b1, W2, b2, W3, b3):
    import ml_dtypes
    f = np.float32
    f8 = ml_dtypes.float8_e4m3
    dt = float(np.asarray(t, dtype=f)[1] - np.asarray(t, dtype=f)[0])
    W1, W2 = np.asarray(W1, f), np.asarray(W2, f)
    W3dt = dt * np.asarray(W3, f)
    s1, s2, s3 = _pow2_scale(W1), _pow2_scale(W2), _pow2_scale(W3dt)

    laug = np.concatenate([np.eye(DIN, dtype=f),
                           np.asarray(W_aug, f).T], axis=1)
    baug = np.concatenate([np.zeros(DIN, f),
                           np.asarray(b_aug, f)]).reshape(STATE, 1)
    w0t = np.ascontiguousarray(np.asarray(W0, f).T)
    w1s = np.ascontiguousarray(_swi_pairs((W1 * s1).T)).astype(f8)
    w2s = np.ascontiguousarray(_swi_pairs((W2 * s2).T)).astype(f8)
    w3s = np.ascontiguousarray(
        _swi_pairs((W3dt * s3).T)[:, :, 0]).astype(f8)  # [128, KP, 2, 128]
    b0r = np.ascontiguousarray(np.asarray(b0, f).reshape(KC, 128).T)
    b1r = np.ascontiguousarray(np.asarray(b1, f).reshape(KC, 128).T)
    b2r = np.ascontiguousarray(np.asarray(b2, f).reshape(KC, 128).T)
    b3dt = (dt * np.asarray(b3, f)).reshape(STATE, 1)
    idt = np.eye(STATE, dtype=f) * s3

    shared = dict(laug=laug, baug=baug, w0t=w0t, w1s=w1s, w2s=w2s, w3s=w3s,
                  b0=b0r, b1=b1r, b2=b2r, b3dt=b3dt, idt=idt)
    in_maps = []
    for c in range(NCORES):
        y0c = np.ascontiguousarray(
            np.asarray(y0, f)[c * BSHARD:(c + 1) * BSHARD]
            .reshape(NTOK, DIN).T)
        in_maps.append(dict(y0t=y0c, **shared))
    return in_maps, (s1, s2, s3)


def _run(inputs, trace=False, **trace_kwargs):
    from concourse.bass_utils import run_bass_kernel_spmd

    in_maps, scales = _make_in_maps(**inputs)
    nc = _build(scales)
    res = run_bass_kernel_spmd(nc, in_maps, core_ids=list(range(NCORES)),
                               trace=trace, **trace_kwargs)
    outs = [res.results[c]["out"] for c in range(NCORES)]
    full = np.concatenate(
        [o.T.reshape(BSHARD, S, DIN) for o in outs], axis=0)
    return np.ascontiguousarray(full, dtype=np.float32), res


def kernel(**inputs):
    out, _ = _run(inputs, trace=False)
    return out


# revision 12
# speedup vs baseline: 1.1948x; 1.0045x over previous
"""Augmented Neural ODE kernel for 8 TRN2 NeuronCores — fp8 SwInterleave variant.

Data-parallel over the batch dim (8 batches/core -> 512 tokens/core);
state kept feature-major [STATE=128 partitions, 512 tokens] in SBUF.
Layers 1-3 (contraction 1024) run as fp8e4m3 DoubleRowSwInterleave
matmuls: weights pre-interleaved host-side ([A127,B127,A126,B126,...]
per partition) so LDWEIGHTS reads contiguously — plain DoubleRow's
gather pays ~35ns/MM of extra PE-array time on top of the 216ns N=512
stream; SwInterleave removes it (measured 251 -> 220 ns/MM back-to-back).
Layer 0 runs in f32r straight off the carry (K=128 can't DoubleRow).
The Euler carry y' = y + dt*f rides layer 3's PSUM accumulation group
via an s3-scaled f32r identity matmul (s3 is the power-of-two fp8
scale of dt*W3, so the fold is lossless).

Note: the chip's power manager caps sustained density — schedules that
would finish below ~660us get every core-domain clock cut to 5/6 (PE
2.4->2.0GHz), which is a net loss. The m-outer emission order here
measures at full clock; denser hand schedules measured faster-per-clock
but throttled slower end-to-end.
"""

import sys

if "/opt/trn_rl_repo" not in sys.path:
    sys.path.insert(0, "/opt/trn_rl_repo")

import numpy as np

B, S, DIN, DAUG = 64, 64, 64, 64
STATE = DIN + DAUG          # 128
HID = 1024
T = 32
NCORES = 8
BSHARD = B // NCORES        # 8
NTOK = BSHARD * S           # 512 tokens per core
KC = HID // 128             # 8 chunks of the hidden dim
KP = KC // 2                # 4 chunk-pairs for DoubleRow

_cached = {}


def _build(scales):
    """scales = (s1, s2, s3) power-of-two per-matrix weight scales."""
    if scales in _cached:
        return _cached[scales]
    s1, s2, s3 = scales

    import concourse.tile as tile
    from concourse import bacc, mybir

    f32 = mybir.dt.float32
    f32r = mybir.dt.float32r
    fp8 = mybir.dt.float8e4
    SWI = mybir.MatmulPerfMode.DoubleRowSwInterleave
    Tanh = mybir.ActivationFunctionType.Tanh

    nc = bacc.Bacc("TRN2", target_bir_lowering=False, debug=False,
                   num_devices=NCORES)

    y0t_d = nc.dram_tensor("y0t", [DIN, NTOK], f32r, kind="ExternalInput").ap()
    laug_d = nc.dram_tensor("laug", [DIN, STATE], f32r, kind="ExternalInput").ap()
    baug_d = nc.dram_tensor("baug", [STATE, 1], f32, kind="ExternalInput").ap()
    w0t_d = nc.dram_tensor("w0t", [STATE, HID], f32r, kind="ExternalInput").ap()
    w1s_d = nc.dram_tensor("w1s", [128, KP, KC, 2, 128], fp8, kind="ExternalInput").ap()
    w2s_d = nc.dram_tensor("w2s", [128, KP, KC, 2, 128], fp8, kind="ExternalInput").ap()
    w3s_d = nc.dram_tensor("w3s", [128, KP, 2, STATE], fp8, kind="ExternalInput").ap()
    b0_d = nc.dram_tensor("b0", [128, KC], f32, kind="ExternalInput").ap()
    b1_d = nc.dram_tensor("b1", [128, KC], f32, kind="ExternalInput").ap()
    b2_d = nc.dram_tensor("b2", [128, KC], f32, kind="ExternalInput").ap()
    b3dt_d = nc.dram_tensor("b3dt", [STATE, 1], f32, kind="ExternalInput").ap()
    idt_d = nc.dram_tensor("idt", [STATE, STATE], f32r, kind="ExternalInput").ap()
    out_d = nc.dram_tensor("out", [DIN, NTOK], f32r, kind="ExternalOutput").ap()

    with tile.TileContext(nc) as tc:
        with tc.tile_pool(name="wpool", bufs=1) as wpool, \
             tc.tile_pool(name="hpool", bufs=12) as hpool, \
             tc.tile_pool(name="ypool", bufs=2) as ypool, \
             tc.tile_pool(name="pspool", bufs=8, space="PSUM") as pspool:

            laug = wpool.tile([DIN, STATE], f32r)
            nc.scalar.dma_start(laug[:], laug_d[:])
            y0t = wpool.tile([DIN, NTOK], f32r)
            nc.sync.dma_start(y0t[0:DIN // 2], y0t_d[0:DIN // 2])
            nc.gpsimd.dma_start(y0t[DIN // 2:], y0t_d[DIN // 2:])
            baug = wpool.tile([128, 1], f32)
            nc.scalar.dma_start(baug[:], baug_d[:])
            w0t = wpool.tile([128, HID], f32r)
            nc.sync.dma_start(w0t[:, 0:HID // 2], w0t_d[:, 0:HID // 2])
            nc.gpsimd.dma_start(w0t[:, HID // 2:], w0t_d[:, HID // 2:])

            w1s = wpool.tile([128, KP, KC, 2, 128], fp8)
            nc.sync.dma_start(w1s[:, 0], w1s_d[:, 0])
            nc.gpsimd.dma_start(w1s[:, 1], w1s_d[:, 1])
            nc.scalar.dma_start(w1s[:, 2], w1s_d[:, 2])
            nc.gpsimd.dma_start(w1s[:, 3], w1s_d[:, 3])
            w2s = wpool.tile([128, KP, KC, 2, 128], fp8)
            nc.sync.dma_start(w2s[:, 0], w2s_d[:, 0])
            nc.gpsimd.dma_start(w2s[:, 1], w2s_d[:, 1])
            nc.scalar.dma_start(w2s[:, 2], w2s_d[:, 2])
            nc.sync.dma_start(w2s[:, 3], w2s_d[:, 3])
            w3s = wpool.tile([128, KP, 2, STATE], fp8)
            nc.gpsimd.dma_start(w3s[:], w3s_d[:])
            idt = wpool.tile([STATE, STATE], f32r)
            nc.scalar.dma_start(idt[:], idt_d[:])
            b0 = wpool.tile([128, KC], f32)
            nc.sync.dma_start(b0[:], b0_d[:])
            b1 = wpool.tile([128, KC], f32)
            nc.sync.dma_start(b1[:], b1_d[:])
            b2 = wpool.tile([128, KC], f32)
            nc.sync.dma_start(b2[:], b2_d[:])
            b3dt = wpool.tile([128, 1], f32)
            nc.sync.dma_start(b3dt[:], b3dt_d[:])

            # augment: y = [y0; W_aug y0 + b_aug]   (K = 64, one-time)
            ps = pspool.tile([128, NTOK], f32, tag="ps")
            nc.tensor.matmul(ps[:], lhsT=laug[:], rhs=y0t[:],
                             start=True, stop=True)
            y = ypool.tile([128, NTOK], f32r, tag="y")
            nc.vector.tensor_scalar(y[:], ps[:], 1.0, baug[:, 0:1],
                                    mybir.AluOpType.mult,
                                    mybir.AluOpType.add)

            for _step in range(T - 1):
                # layer 0: f32r straight off the carry y
                h0 = [hpool.tile([128, 2, NTOK], fp8, tag="h", name=f"h0_{_step}_{i}")
                      for i in range(KP)]
                for m in range(KC):
                    ps = pspool.tile([128, NTOK], f32, tag="ps")
                    nc.tensor.matmul(ps[:], lhsT=w0t[:, m * 128:(m + 1) * 128],
                                     rhs=y[:], start=True, stop=True)
                    nc.scalar.activation(h0[m // 2][:, m % 2, :], ps[:], Tanh,
                                         bias=b0[:, m:m + 1])
                # layer 1: fp8 SwInterleave, K=256 per matmul
                h1 = [hpool.tile([128, 2, NTOK], fp8, tag="h", name=f"h1_{_step}_{i}")
                      for i in range(KP)]
                for m in range(KC):
                    ps = pspool.tile([128, NTOK], f32, tag="ps")
                    for k in range(KP):
                        nc.tensor.matmul(ps[:], lhsT=w1s[:, k, m],
                                         rhs=h0[k][:],
                                         start=(k == 0), stop=(k == KP - 1),
                                         perf_mode=SWI)
                    nc.scalar.activation(h1[m // 2][:, m % 2, :], ps[:], Tanh,
                                         bias=b1[:, m:m + 1], scale=1.0 / s1)
                # layer 2 with layer 3's matmuls interleaved as their h2
                # pairs become ready; the Euler carry rides the same PSUM
                # group via the s3-scaled f32r identity matmul
                h2 = [hpool.tile([128, 2, NTOK], fp8, tag="h", name=f"h2_{_step}_{i}")
                      for i in range(KP)]
                ps3 = pspool.tile([128, NTOK], f32, tag="ps", name=f"ps3_{_step}")
                nc.tensor.matmul(ps3[:], lhsT=idt[:], rhs=y[:],
                                 start=True, stop=False)
                for m in range(KC):
                    ps = pspool.tile([128, NTOK], f32, tag="ps")
                    for k in range(KP):
                        nc.tensor.matmul(ps[:], lhsT=w2s[:, k, m],
                                         rhs=h1[k][:],
                                         start=(k == 0), stop=(k == KP - 1),
                                         perf_mode=SWI)
                    nc.scalar.activation(h2[m // 2][:, m % 2, :], ps[:], Tanh,
                                         bias=b2[:, m:m + 1], scale=1.0 / s2)
                    if m == 3 or m == 5 or m == 7:
                        k = (m - 3) // 2
                        nc.tensor.matmul(ps3[:], lhsT=w3s[:, k],
                                         rhs=h2[k][:],
                                         start=False, stop=False,
                                         perf_mode=SWI)
                nc.tensor.matmul(ps3[:], lhsT=w3s[:, 3], rhs=h2[3][:],
                                 start=False, stop=True, perf_mode=SWI)
                # carry on the vector engine; scalar stays free for tanhs
                y = ypool.tile([128, NTOK], f32r, tag="y")
                nc.vector.tensor_scalar(y[:], ps3[:], 1.0 / s3, b3dt[:, 0:1],
                                        mybir.AluOpType.mult,
                                        mybir.AluOpType.add)

            nc.sync.dma_start(out_d[:], y[0:DIN, :])

    nc.compile()
    _cached[scales] = nc
    return nc


def _pow2_scale(W, target=224.0):
    import math
    return 2.0 ** math.floor(math.log2(target / float(np.abs(W).max())))


def _swi_pairs(Wt):
    """Wt: [K_in, M_out] (lhsT orientation) with K_in = 256*kp.
    Returns [128, kp, M_out//128, 2, 128] in SwInterleave layout:
    per partition the 256 weights of a (k, m) chunk are
    [A_{127}, B_{127}, ..., A_0, B_0] with A/B = K-subchunks 2k/2k+1
    and columns reversed."""
    K_in, M_out = Wt.shape
    kp = K_in // 256
    mc = M_out // 128
    out = np.empty((128, kp, mc, 2, 128), np.float32)
    for k in range(kp):
        lo = Wt[(2 * k) * 128:(2 * k + 1) * 128]
        hi = Wt[(2 * k + 1) * 128:(2 * k + 2) * 128]
        for m in range(mc):
            ms = slice(m * 128, (m + 1) * 128)
            pair = np.stack([lo[:, ms], hi[:, ms]], axis=1)  # [128, 2, 128]
            tmp = pair[:, :, ::-1].transpose(0, 2, 1)        # [128, 128, 2]
            out[:, k, m] = tmp.reshape(128, 2, 128)
    return out


def _make_in_maps(y0, t, W_aug, b_aug, W0, b0, W1, b1, W2, b2, W3, b3):
    import ml_dtypes
    f = np.float32
    f8 = ml_dtypes.float8_e4m3
    dt = float(np.asarray(t, dtype=f)[1] - np.asarray(t, dtype=f)[0])
    W1, W2 = np.asarray(W1, f), np.asarray(W2, f)
    W3dt = dt * np.asarray(W3, f)
    s1, s2, s3 = _pow2_scale(W1), _pow2_scale(W2), _pow2_scale(W3dt)

    laug = np.concatenate([np.eye(DIN, dtype=f),
                           np.asarray(W_aug, f).T], axis=1)
    baug = np.concatenate([np.zeros(DIN, f),
                           np.asarray(b_aug, f)]).reshape(STATE, 1)
    w0t = np.ascontiguousarray(np.asarray(W0, f).T)
    w1s = np.ascontiguousarray(_swi_pairs((W1 * s1).T)).astype(f8)
    w2s = np.ascontiguousarray(_swi_pairs((W2 * s2).T)).astype(f8)
    w3s = np.ascontiguousarray(
        _swi_pairs((W3dt * s3).T)[:, :, 0]).astype(f8)  # [128, KP, 2, 128]
    b0r = np.ascontiguousarray(np.asarray(b0, f).reshape(KC, 128).T)
    b1r = np.ascontiguousarray(np.asarray(b1, f).reshape(KC, 128).T)
    b2r = np.ascontiguousarray(np.asarray(b2, f).reshape(KC, 128).T)
    b3dt = (dt * np.asarray(b3, f)).reshape(STATE, 1)
    idt = np.eye(STATE, dtype=f) * s3

    shared = dict(laug=laug, baug=baug, w0t=w0t, w1s=w1s, w2s=w2s, w3s=w3s,
                  b0=b0r, b1=b1r, b2=b2r, b3dt=b3dt, idt=idt)
    in_maps = []
    for c in range(NCORES):
        y0c = np.ascontiguousarray(
            np.asarray(y0, f)[c * BSHARD:(c + 1) * BSHARD]
            .reshape(NTOK, DIN).T)
        in_maps.append(dict(y0t=y0c, **shared))
    return in_maps, (s1, s2, s3)


def _run(inputs, trace=False, **trace_kwargs):
    from concourse.bass_utils import run_bass_kernel_spmd

    in_maps, scales = _make_in_maps(**inputs)
    nc = _build(scales)
    res = run_bass_kernel_spmd(nc, in_maps, core_ids=list(range(NCORES)),
                               trace=trace, **trace_kwargs)
    outs = [res.results[c]["out"] for c in range(NCORES)]
    full = np.concatenate(
        [o.T.reshape(BSHARD, S, DIN) for o in outs], axis=0)
    return np.ascontiguousarray(full, dtype=np.float32), res


def kernel(**inputs):
    out, _ = _run(inputs, trace=False)
    return out
